# revision 20
# baseline (speedup 1.0000x reference)
"""Physics-attention (slice attention) Trainium2 kernel, 8-core SPMD.

Problem: out = PhysicsAttention(x) with B=4, N=32768, DIM=256, H=8, Ch=64, M=64.

Sharding: core i handles batch b=i//2, token half i%2 (T=16384 tokens/core).
The N->M reductions (slice_token, slice weight column sums) are AllReduced
over core pairs [[0,1],[2,3],[4,5],[6,7]].

Device pipeline per core (all big matmuls bf16 with fp32 PSUM accumulation):
  pass 1 (per 128-token tile):
    logits = x @ Wcomb.T + bcomb   (Wcomb = (Wslice@Wx_h)/temp_h, host-fused;
                                    bias via PSUM-preload matmul)
    fx     = x @ Wfx.T             (bfx folded in post-AR as rank-1 update)
    e = exp(logits); esum (per-head strided reduce); sw = e / esum
    st_raw[(h,m),c] += sw.T @ [fx | ones]   (pair-packed, PSUM-accumulated;
                                             ones column yields column sums)
    swT chunks via PE transpose -> persistent SBUF (for pass 2)
  AllReduce(st_raw + colsums) over the pair
  tiny slice attention per head (64x64) -> Wff[(h,m),d] with all scale
    factors folded in
  pass 2 (per 128-token tile):
    out = sw @ Wff + bout  (bout PSUM-preloaded, 4 chunk matmuls accumulated)

Instruction-level constraint honored throughout: engine instructions can
carry at most 2 semaphore waits (DMA transposes only 1, so none are used).
Dependencies are funneled so each matmul/activation sees <=2 producer procs:
constants arrive in one blob DMA, softmax bias uses a DVE-written zero tile,
the fx ones-columns are pre-set once per slot by the DVE, and Wff chunks
bounce through DRAM so each has a single writer.
"""

import os
import sys

sys.path.insert(0, "/opt/trn_rl_repo")

import numpy as np
import ml_dtypes

import concourse.bass as bass
import concourse.bacc as bacc
import concourse.bass_isa as bass_isa
import concourse.mybir as mybir
import concourse.tile as tile
from concourse import bass_utils

BF16 = mybir.dt.bfloat16
F32 = mybir.dt.float32

B, N, DIM = 4, 32768, 256
H, CH, M = 8, 64, 64
INNER = H * CH          # 512
HM = H * M              # 512
T = int(os.environ.get("KERNEL_T", "16384"))   # tokens per core
NT = T // 128           # token tiles
NSUP = max(1, T // 2048)  # x super-tiles
SUP = T // NSUP         # tokens per super-tile

# const blob layout (bf16, [128, CBLOB]):
#   [0:1024)     wc chunks (c*512)
#   [1024:2048)  wf chunks
#   [2048:2176)  identity
#   row 0 [2176:2304)  ones(128)
#   row 0 [2304:2816)  bcomb
CBLOB = 3072
# attn blob (bf16, [64, ABLOB]): wqt 0:64, wkt 64:128, wvt 128:192,
#   woutt[c, h*256:(h+1)*256] at 192+...
ABLOB = 192 + H * DIM
# fp32 blob [128, FBLOB]: bfxb (rows 0-63 used); bout added on host
FBLOB = INNER

_PROGRAM_CACHE = {}


def _build_program(no_collective=False):
    nc = bacc.Bacc(
        "TRN2",
        target_bir_lowering=False,
        debug=False,
        num_devices=None if no_collective else 8,
        dynamic_dma_scratch_size=8192,
    )

    # ---- I/O ----
    xt_d = nc.dram_tensor("xt", [DIM, T], BF16, kind="ExternalInput")
    cb_d = nc.dram_tensor("cblob", [128, CBLOB], BF16, kind="ExternalInput")
    ab_d = nc.dram_tensor("ablob", [64, ABLOB], BF16, kind="ExternalInput")
    fb_d = nc.dram_tensor("fblob", [128, FBLOB], F32, kind="ExternalInput")
    out_d = nc.dram_tensor("out", [T, DIM], F32, kind="ExternalOutput")

    AluOp = mybir.AluOpType
    ActFn = mybir.ActivationFunctionType

    with tile.TileContext(nc) as tc:
        with (
            tc.tile_pool(name="const", bufs=1) as cpool,
            tc.tile_pool(name="swt", bufs=1) as swtpool,
            tc.tile_pool(name="xts", bufs=2) as xtspool,
            tc.tile_pool(name="work", bufs=3) as work,
            tc.tile_pool(name="attn", bufs=1) as attn,
            tc.tile_pool(name="psum", bufs=1, space="PSUM") as psum,
            tc.tile_pool(name="dram", bufs=1, space="DRAM") as dram,
        ):
            # ---- constants: one DMA per blob ----
            cb = cpool.tile([128, CBLOB], BF16)
            nc.gpsimd.dma_start(cb[:], cb_d[:])
            ab = cpool.tile([64, ABLOB], BF16)
            nc.gpsimd.dma_start(ab[:], ab_d[:])
            fb = cpool.tile([128, FBLOB], F32)
            nc.gpsimd.dma_start(fb[:], fb_d[:])

            def wc(c):
                return cb[:, c * 512:(c + 1) * 512]

            def wfc(c):
                return cb[:, 1024 + c * 512:1024 + (c + 1) * 512]

            ident = cb[:, 2048:2176]
            ones1 = cb[0:1, 2176:2304]
            bcomb = cb[0:1, 2304:2816]
            wqt = ab[:, 0:64]
            wkt = ab[:, 64:128]
            wvt = ab[:, 128:192]

            def woutt(h):
                return ab[:, 192 + h * DIM:192 + (h + 1) * DIM]

            bfxb = fb[0:64, 0:INNER]

            # fx tiles with persistent ones-columns (explicit triple buffer)
            fxs_t = []
            for j in range(3):
                ft = cpool.tile([128, H, CH + 1], BF16, name=f"fxs{j}")
                nc.vector.memset(ft[:, :, CH], 1.0)
                fxs_t.append(ft)

            # persistent transposed slice weights [hm, n] as 4 chunks
            swt = swtpool.tile([128, 4, T], BF16)

            # preamble-allocated late-phase tiles: claiming SBUF now (with a
            # touch) avoids slot-recycling deps that overflow the 2-wait
            # budget of engine instructions in the attn/pass-2 phases
            def pre(name, shape, dtype):
                t = cpool.tile(shape, dtype, name=name)
                nc.vector.memset(t[0:shape[0], 0:1] if len(shape) == 2
                                 else t[0:shape[0], 0:1, 0:1], 0.0)
                return t

            stage = pre("stage", [128, 4, 130], F32)
            arq = pre("arq", [128, 4, 130], F32)
            arq2 = pre("arq2", [64, 4, 130], F32)
            at_ss1 = pre("at_ss1", [64, 1], F32)
            at_inv = pre("at_inv", [64, 1], F32)
            at_stf = pre("at_stf", [64, CH], F32)
            at_stfb = pre("at_stfb", [64, CH], BF16)
            at_stT = pre("at_stT", [64, CH], BF16)
            at_qT = pre("at_qT", [64, CH], BF16)
            at_kT = pre("at_kT", [64, CH], BF16)
            at_v = pre("at_v", [64, CH], BF16)
            at_pE = pre("at_pE", [64, M], BF16)
            at_prs = pre("at_prs", [64, 1], F32)
            at_prr = pre("at_prr", [64, 1], F32)
            at_pP = pre("at_pP", [64, M], BF16)
            at_pT = pre("at_pT", [64, M], BF16)
            at_otT = pre("at_otT", [CH, M], BF16)
            at_wffh = pre("at_wffh", [64, DIM], BF16)
            wff = [pre(f"wff{c}", [128, DIM], BF16) for c in range(4)]
            outs_t = [pre(f"outs{j}", [128, DIM], F32) for j in range(3)]
            e_t_r = [pre(f"e{j}", [128, HM], BF16) for j in range(3)]
            es_t_r = [pre(f"es{j}", [128, H], F32) for j in range(3)]
            r_t_r = [pre(f"r{j}", [128, H], F32) for j in range(3)]
            sw_t_r = [pre(f"sw{j}", [128, HM], BF16) for j in range(3)]

            # ---- pass 1 ----
            if True:
                ps_lg = ps_fx = ps_st = ps_xp = psum
                st_p = [
                    ps_st.tile([128, 130], F32, tag=f"st{j}", name=f"st{j}")
                    for j in range(4)
                ]

                def stage_b(t):
                    # consumers of sw(t)/fx(t): slice-token accumulation and
                    # the sw transpose. Emitted one iteration late so the PE
                    # never stalls on the softmax chain of the current tile.
                    sw_t = sw_t_r[t % 3]
                    fxs = fxs_t[t % 3]
                    first = t == 0
                    last = t == NT - 1
                    for p in range(4):
                        nc.tensor.matmul(
                            st_p[p][:],
                            sw_t[:, p * 128:(p + 1) * 128],
                            fxs[:, 2 * p:2 * p + 2, :],
                            start=first, stop=last,
                        )
                    xp_p = ps_xp.tile([128, 4, 128], BF16, tag="xp", bufs=1,
                                      name=f"xp{t}")
                    for c in range(4):
                        nc.tensor.transpose(
                            xp_p[:, c, :],
                            sw_t[:, c * 128:(c + 1) * 128],
                            ident,
                        )
                    if t % 2 == 0:
                        nc.scalar.copy(swt[:, :, t * 128:(t + 1) * 128], xp_p[:])
                    else:
                        nc.vector.tensor_copy(
                            swt[:, :, t * 128:(t + 1) * 128], xp_p[:])

                for s in range(NSUP):
                    xts = xtspool.tile([128, 2, SUP], BF16, tag="xts")
                    nc.sync.dma_start(
                        xts[:],
                        xt_d[:, s * SUP:(s + 1) * SUP].rearrange(
                            "(c p) n -> p c n", c=2
                        ),
                    )
                    for i in range(SUP // 128):
                        t_idx = s * (SUP // 128) + i
                        n0 = i * 128

                        lg_p = ps_lg.tile([128, HM], F32, tag="lgfx", bufs=3)
                        fx_p = ps_fx.tile([128, INNER], F32, tag="lgfx", bufs=3)
                        # bias preload + projections
                        nc.tensor.matmul(lg_p[:], ones1, bcomb,
                                         start=True, stop=False)
                        for c in range(2):
                            xc = xts[:, c, n0:n0 + 128]
                            nc.tensor.matmul(lg_p[:], xc, wc(c),
                                             start=False, stop=(c == 1))
                            nc.tensor.matmul(fx_p[:], xc, wfc(c),
                                             start=(c == 0), stop=(c == 1))

                        # softmax over m within each head (free-axis groups)
                        e_t = e_t_r[t_idx % 3]
                        nc.scalar.activation(e_t[:], lg_p[:], ActFn.Exp)
                        es_t = es_t_r[t_idx % 3]
                        nc.vector.tensor_reduce(
                            es_t[:],
                            e_t[:].rearrange("p (h m) -> p h m", h=H),
                            axis=mybir.AxisListType.X,
                            op=AluOp.add,
                        )
                        r_t = r_t_r[t_idx % 3]
                        nc.vector.reciprocal(r_t[:], es_t[:])
                        sw_t = sw_t_r[t_idx % 3]
                        nc.vector.tensor_tensor(
                            sw_t[:].rearrange("p (h m) -> p h m", h=H),
                            e_t[:].rearrange("p (h m) -> p h m", h=H),
                            r_t[:].unsqueeze(-1).broadcast_to([128, H, M]),
                            op=AluOp.mult,
                        )

                        # fx to bf16 (ones columns pre-set in the fxs tiles)
                        fxs = fxs_t[t_idx % 3]
                        nc.scalar.copy(
                            fxs[:, :, 0:CH],
                            fx_p[:].rearrange("p (h c) -> p h c", h=H),
                        )

                        if t_idx > 1:
                            stage_b(t_idx - 2)
                stage_b(NT - 2)
                stage_b(NT - 1)

                # ---- stage slice-token accumulators + AllReduce ----
                stage = attn.tile([128, 4, 130], F32)
                for j in range(2):
                    nc.scalar.copy(stage[:, 2 * j:2 * j + 2, :], st_p[j][:])

            ar_in = dram.tile([128, 4 * 130], F32)
            ar_out = dram.tile([128, 4 * 130], F32)
            nc.gpsimd.dma_start(ar_in[:], stage[:].rearrange("p a b -> p (a b)"))
            if no_collective:
                nc.gpsimd.dma_start(ar_out[:], ar_in[:])
            else:
                nc.gpsimd.collective_compute(
                    "AllReduce",
                    AluOp.add,
                    replica_groups=[[0, 1], [2, 3], [4, 5], [6, 7]],
                    ins=[ar_in[:].opt()],
                    outs=[ar_out[:].opt()],
                )
            arq = attn.tile([128, 4, 130], F32)
            nc.gpsimd.dma_start(arq[:].rearrange("p a b -> p (a b)"), ar_out[:])
            # odd heads live on partitions 64-127; shift them down so every
            # per-head op runs lane-aligned on partitions 0-63
            arq2 = attn.tile([64, 4, 130], F32)
            nc.gpsimd.dma_start(arq2[:], arq[64:128, :, :])

            # ---- tiny slice attention per head -> Wff chunks ----
            wff_dram = dram.tile([4, 128, DIM], BF16)
            with tc.tile_pool(name="ps_at", bufs=2, space="PSUM") as ps_at:
                for h in range(H):
                    pr = h // 2
                    r0 = (h % 2) * 64
                    c0 = (h % 2) * 65
                    src = arq if h % 2 == 0 else arq2
                    st_raw = src[0:64, pr, c0:c0 + 64]      # [64m, 64c]
                    ssum = src[0:64, pr, c0 + 64:c0 + 65]   # [64m, 1]

                    ss1 = attn.tile([64, 1], F32, tag="ss1")
                    nc.vector.tensor_scalar_add(ss1[:], ssum, 1e-5)
                    inv = attn.tile([64, 1], F32, tag="inv")
                    nc.vector.reciprocal(inv[:], ss1[:])

                    # st_tok*(ssum+eps) = st_raw + ssum*bfx ; *inv deferred
                    stf = attn.tile([64, CH], F32, tag="stf")
                    nc.vector.scalar_tensor_tensor(
                        stf[:],
                        bfxb[:, h * CH:(h + 1) * CH],
                        ssum,
                        st_raw,
                        op0=AluOp.mult,
                        op1=AluOp.add,
                    )
                    stfb = attn.tile([64, CH], BF16, tag="stfb")
                    nc.vector.tensor_scalar_mul(stfb[:], stf[:], inv[:])

                    stT_p = ps_at.tile([64, CH], BF16, tag="xp", bufs=1)
                    nc.tensor.transpose(stT_p[:], stfb[:], ident[0:64, 0:64])
                    stT = attn.tile([64, CH], BF16, tag="stT")
                    nc.scalar.copy(stT[:], stT_p[:])

                    qT_p = ps_at.tile([64, CH], F32, tag="lgfx", bufs=3)
                    nc.tensor.matmul(qT_p[:], wqt, stT[:])
                    qT = attn.tile([64, CH], BF16, tag="qT")
                    nc.scalar.copy(qT[:], qT_p[:])
                    kT_p = ps_at.tile([64, CH], F32, tag="lgfx", bufs=3)
                    nc.tensor.matmul(kT_p[:], wkt, stT[:])
                    kT = attn.tile([64, CH], BF16, tag="kT")
                    nc.scalar.copy(kT[:], kT_p[:])
                    v_p = ps_at.tile([64, CH], F32, tag="lgfx", bufs=3)
                    nc.tensor.matmul(v_p[:], stT[:], wvt)
                    v_t = attn.tile([64, CH], BF16, tag="v")
                    nc.scalar.copy(v_t[:], v_p[:])

                    s_p = ps_at.tile([64, M], F32, tag="lgfx", bufs=3)
                    nc.tensor.matmul(s_p[:], qT[:], kT[:])
                    pE = attn.tile([64, M], BF16, tag="pE")
                    prs = attn.tile([64, 1], F32, tag="prs")
                    nc.scalar.activation(pE[:], s_p[:], ActFn.Exp,
                                         accum_out=prs[:])
                    prr = attn.tile([64, 1], F32, tag="prr")
                    nc.vector.reciprocal(prr[:], prs[:])
                    pP = attn.tile([64, M], BF16, tag="pP")
                    nc.vector.tensor_scalar_mul(pP[:], pE[:], prr[:])

                    pT_p = ps_at.tile([64, M], BF16, tag="xp", bufs=1)
                    nc.tensor.transpose(pT_p[:], pP[:], ident[0:64, 0:64])
                    pT = attn.tile([64, M], BF16, tag="pT")
                    nc.scalar.copy(pT[:], pT_p[:])

                    ot_p = ps_at.tile([CH, M], F32, tag="lgfx", bufs=3)
                    nc.tensor.matmul(ot_p[:], v_t[:], pT[:])
                    otT = attn.tile([CH, M], BF16, tag="otT")
                    nc.scalar.copy(otT[:], ot_p[:])

                    wf_p = ps_at.tile([M, DIM], F32, tag="lgfx", bufs=3)
                    nc.tensor.matmul(wf_p[:], otT[:], woutt(h))
                    wffh = attn.tile([64, DIM], BF16, tag="wffh")
                    nc.vector.tensor_scalar_mul(wffh[:], wf_p[:], inv[:])
                    nc.gpsimd.dma_start(wff_dram[h // 2, r0:r0 + 64, :], wffh[:])

            # single-writer reload of the assembled Wff chunks
            wff = []
            for c in range(4):
                wt = attn.tile([128, DIM], BF16, name=f"wff{c}")
                nc.gpsimd.dma_start(wt[:], wff_dram[c])
                wff.append(wt)

            # ---- pass 2 ----
            if True:
                for t_idx in range(NT):
                    n0 = t_idx * 128
                    op_p = psum.tile([128, DIM], F32, tag=f"st{t_idx % 2}", bufs=1)
                    for c in range(4):
                        nc.tensor.matmul(
                            op_p[:],
                            swt[:, c, n0:n0 + 128],
                            wff[c][:],
                            start=(c == 0), stop=(c == 3),
                        )
                    outs = work.tile([128, DIM], F32, tag="outs")
                    nc.scalar.copy(outs[:], op_p[:])
                    nc.sync.dma_start(out_d[n0:n0 + 128, :], outs[:])

    return nc


def _host_prep(inputs):
    bf16 = ml_dtypes.bfloat16
    x = np.asarray(inputs["x"], dtype=np.float32)
    Wfx = np.asarray(inputs["Wfx"], dtype=np.float32)
    bfx = np.asarray(inputs["bfx"], dtype=np.float32)
    Wx = np.asarray(inputs["Wx"], dtype=np.float32)
    bx = np.asarray(inputs["bx"], dtype=np.float32)
    Wsl = np.asarray(inputs["Wslice"], dtype=np.float32)
    bsl = np.asarray(inputs["bslice"], dtype=np.float32)
    temp = np.asarray(inputs["temperature"], dtype=np.float32)
    Wq = np.asarray(inputs["Wq"], dtype=np.float32)
    Wk = np.asarray(inputs["Wk"], dtype=np.float32)
    Wv = np.asarray(inputs["Wv"], dtype=np.float32)
    Wout = np.asarray(inputs["Wout"], dtype=np.float32)
    bout = np.asarray(inputs["bout"], dtype=np.float32)

    t = np.clip(temp, 0.1, 5.0).reshape(H)
    Wxh = Wx.reshape(H, CH, DIM)
    Wcomb = (np.einsum("mc,hcd->hmd", Wsl, Wxh) / t[:, None, None]).reshape(HM, DIM)
    bcomb = ((bsl[None, :] + np.einsum("mc,hc->hm", Wsl, bx.reshape(H, CH)))
             / t[:, None]).reshape(HM)

    cblob = np.zeros((128, CBLOB), dtype=bf16)
    wcombt = np.ascontiguousarray(Wcomb.T).astype(bf16)    # [256, 512]
    wfxt = np.ascontiguousarray(Wfx.T).astype(bf16)        # [256, 512]
    for c in range(2):
        cblob[:, c * 512:(c + 1) * 512] = wcombt[c * 128:(c + 1) * 128, :]
        cblob[:, 1024 + c * 512:1024 + (c + 1) * 512] = wfxt[c * 128:(c + 1) * 128, :]
    cblob[:, 2048:2176] = np.eye(128, dtype=np.float32).astype(bf16)
    cblob[0, 2176:2304] = np.ones(128, dtype=bf16)
    cblob[0, 2304:2816] = bcomb.astype(bf16)
    cblob[0, 2816:2816 + DIM] = bout.astype(bf16)

    ablob = np.zeros((64, ABLOB), dtype=bf16)
    ablob[:, 0:64] = ((Wq / np.sqrt(CH)).T).astype(bf16)
    ablob[:, 64:128] = (Wk.T).astype(bf16)
    ablob[:, 128:192] = (Wv.T).astype(bf16)
    woutr = Wout.reshape(DIM, H, CH).transpose(2, 1, 0)    # [c, h, d]
    ablob[:, 192:] = woutr.reshape(64, H * DIM).astype(bf16)

    fblob = np.ascontiguousarray(
        np.broadcast_to(bfx.reshape(INNER), (128, INNER))
    ).astype(np.float32)

    shared = {"cblob": cblob, "ablob": ablob, "fblob": fblob}

    in_maps = []
    for core in range(8):
        b, half = core // 2, core % 2
        xs = x[b, half * T:(half + 1) * T, :]          # [T, DIM]
        xt = np.ascontiguousarray(xs.T).astype(bf16)   # [DIM, T]
        m = dict(shared)
        m["xt"] = xt
        in_maps.append(m)
    return in_maps


def kernel(**inputs):
    if "prog" not in _PROGRAM_CACHE:
        _PROGRAM_CACHE["prog"] = _build_program()
    nc = _PROGRAM_CACHE["prog"]

    in_maps = _host_prep(inputs)
    trace = bool(int(os.environ.get("KERNEL_TRACE", "0")))
    res = bass_utils.run_bass_kernel_spmd(
        nc, in_maps, core_ids=list(range(8)), trace=trace
    )
    if trace:
        kernel.last_exec_time_ns = res.exec_time_ns
        kernel.last_results = res

    out = np.empty((B, N, DIM), dtype=np.float32)
    for core in range(8):
        b, half = core // 2, core % 2
        out[b, half * T:(half + 1) * T, :] = res.results[core]["out"]
    out += np.asarray(inputs["bout"], dtype=np.float32)  # exact fp32 bias
    return out
